# revision 14
# baseline (speedup 1.0000x reference)
"""GAT 2-layer + BN + classifier on 8 TRN2 NeuronCores (Bass/Tile).

v3: dst-block edge sharding, host-gathered per-edge attention scalars,
bf16 table/matmuls, self-loops via a strided own-shard load. 5 launches:
  L1   node: h1 = x @ [W1|W1@as|W1@ad] (bf16) -> table shard [h|1] + av
  E(1) edge: per-subtile indirect gather of tab[src] (128 rows/op, the
       TRN2 Q7 descriptor-emission limit), one fused is_equal*w DVE op +
       one bf16 LDW/MM pair per 128-edge subtile accumulating into PSUM;
       softmax denominator from the table's ones column; self-loop term
       fused via scalar_tensor_tensor; per-core BN partial stats
  L3   node: BN1 apply (7 PE transposes into one packed PSUM tile +
       ONE ACT relu(scale*x+bias) per 7-block group) + @W2aug
  E(2) edge: same NEFF as E(1) on table2
  L5   node: BN2 apply + @Wc + bc -> logits shard
Host work is index-only glue: edge sort/shard, per-edge alpha = as[src]+
ad[dst] gather (static indices), table assembly/rotation, weight
preprocessing, pad-node BN2-stats correction.

All inter-launch tensors use p-major block layout:
  row of node (block t, slot p) lives at partition p, column t*W..(t+1)*W
so every regular DMA is 128 large contiguous descriptors. Table rows are
ordered (p*GBLK + b); row of global node g in core c's rolled table is
(g%128)*GBLK + ((g//128 - 98c) % 784).

Perf note (measured): indirect-DMA descriptor emission costs ~8-10ns per
descriptor of serial gpsimd/Q7 time on TRN2 — ~213k per-edge descriptors
per core per edge launch = ~1.9ms, the hard bottleneck. DVE/PE/DMA all
hide under it. dma_gather, multi-SWDGE-queue, and batched indirect were
measured and do not beat it (batched indirect reads contiguous runs, not
per-index rows, on real HW).
"""
import sys
sys.path.insert(0, '/opt/trn_rl_repo')
sys.path.insert(0, '/root/.axon_site')
import numpy as np
import ml_dtypes

import concourse.bass as bass
import concourse.bacc as bacc
import concourse.tile as tile
from concourse import mybir
from concourse.masks import make_identity

F32 = mybir.dt.float32
BF16 = mybir.dt.bfloat16
I32 = mybir.dt.int32
BF = ml_dtypes.bfloat16

N = 100000
NCORE = 8
BLK = 128
NPAD = 100352            # 784 blocks of 128
PC = NPAD // NCORE       # 12544 nodes/core
NBLK = PC // BLK         # 98 dst blocks per core
GBLK = NPAD // BLK       # 784 global blocks
TC = 129                 # table row: [h(128) | 1]
HID = 128
NCLS = 40
NEG = 0.2
EPS = 1e-5
GRP = 7                  # blocks per grouped output DMA (98 = 14*7)
PAD_ALPHA = -1000.0      # exp(lrelu(PAD_ALPHA)) == 0 exactly

_EXEC_NS = []
PROFILE = False
RUN_HOOK = None


def _run(nc, in_maps, label):
    if RUN_HOOK is not None:
        return RUN_HOOK(nc, in_maps, label)
    from concourse import bass2jax
    return bass2jax.run_bass_via_pjrt(nc, in_maps, n_cores=NCORE)


# ---------------------------------------------------------------- L1 node
def build_l1():
    nc = bacc.Bacc("TRN2", target_bir_lowering=False, debug=False, num_devices=NCORE)
    xT = nc.dram_tensor("xT", [128, PC], BF16, kind="ExternalInput")
    waug = nc.dram_tensor("waug", [128, HID + 2], BF16, kind="ExternalInput")
    tab = nc.dram_tensor("tab", [128, NBLK * TC], BF16, kind="ExternalOutput")
    av = nc.dram_tensor("av", [128, NBLK * 2], F32, kind="ExternalOutput")

    with tile.TileContext(nc) as tc:
        with (
            tc.tile_pool(name="c", bufs=1) as cp,
            tc.tile_pool(name="o", bufs=3) as op,
            tc.tile_pool(name="ps", bufs=4, space="PSUM") as pp,
        ):
            w_sb = cp.tile([128, HID + 2], BF16)
            nc.sync.dma_start(w_sb[:], waug[:])
            x_sb = cp.tile([128, PC], BF16)
            nc.sync.dma_start(x_sb[:], xT[:])
            avst = cp.tile([128, NBLK * 2], F32)

            HW2 = HID + 2
            for g0 in range(0, NBLK, GRP):
                ot = op.tile([128, GRP * TC], BF16, tag="o")
                nc.vector.memset(ot[:], 1.0)
                for k0, ln in ((0, 3), (3, 2), (5, 2)):
                    hg = pp.tile([128, 3 * HW2], F32, tag="h")
                    for j in range(ln):
                        t = g0 + k0 + j
                        nc.tensor.matmul(
                            out=hg[:, j * HW2:(j + 1) * HW2],
                            lhsT=x_sb[:, t * 128:(t + 1) * 128],
                            rhs=w_sb[:], start=True, stop=True)
                    ha = hg[:]
                    oa = ot[:]
                    src3 = bass.AP(ha.tensor, ha.offset,
                                   [list(ha.ap[0]), [HW2, ln], [1, HID]])
                    dst3 = bass.AP(oa.tensor, oa.offset + (k0 * TC),
                                   [list(oa.ap[0]), [TC, ln], [1, HID]])
                    if k0 % 2 == 0:
                        nc.scalar.activation(
                            out=dst3, in_=src3,
                            func=mybir.ActivationFunctionType.Copy)
                    else:
                        nc.vector.tensor_copy(out=dst3, in_=src3)
                    sav = bass.AP(ha.tensor, ha.offset + HID,
                                  [list(ha.ap[0]), [HW2, ln], [1, 2]])
                    nc.vector.tensor_copy(
                        out=avst[:, (g0 + k0) * 2:(g0 + k0 + ln) * 2], in_=sav)
                nc.scalar.dma_start(tab[:, g0 * TC:(g0 + GRP) * TC], ot[:])
            nc.sync.dma_start(av[:], avst[:])
    nc.compile()
    return nc


# ---------------------------------------------------------------- edge kernel
def build_edge(t_counts, last_cnt):
    """t_counts: NBLK subtile counts; last_cnt: rows in each block's last
    subtile (max over cores, <=128). Both shared across cores."""
    nsub = int(sum(t_counts))
    TMAX = int(max(t_counts))
    nc = bacc.Bacc("TRN2", target_bir_lowering=False, debug=False, num_devices=NCORE,
                   dynamic_dma_scratch_size=65536)
    tab = nc.dram_tensor("tab", [NPAD, TC], BF16, kind="ExternalInput")
    src_idx = nc.dram_tensor("src_idx", [128, nsub], I32, kind="ExternalInput")
    dst_loc = nc.dram_tensor("dst_loc", [128, nsub], F32, kind="ExternalInput")
    alpha = nc.dram_tensor("alpha", [128, nsub], F32, kind="ExternalInput")
    alphasl = nc.dram_tensor("alphasl", [128, NBLK], F32, kind="ExternalInput")
    agg = nc.dram_tensor("agg", [128, NBLK * HID], BF16, kind="ExternalOutput")
    stats = nc.dram_tensor("stats", [1, 256], F32, kind="ExternalOutput")

    with tile.TileContext(nc) as tc:
        with (
            tc.tile_pool(name="c", bufs=1) as cp,
            tc.tile_pool(name="g", bufs=8) as gp,
            tc.tile_pool(name="sw", bufs=8) as swp,
            tc.tile_pool(name="sq", bufs=3) as sqp,
            tc.tile_pool(name="rc", bufs=6) as rcp,
            tc.tile_pool(name="og", bufs=2) as ogp,
            tc.tile_pool(name="pb", bufs=2, space="PSUM") as pbp,
            tc.tile_pool(name="pst", bufs=1, space="PSUM") as pstp,
        ):
            iota_i = cp.tile([128, 128], I32)
            nc.gpsimd.iota(iota_i[:], pattern=[[1, 128]], base=0, channel_multiplier=0)
            iota_b = cp.tile([128, 128], BF16)
            nc.vector.tensor_copy(out=iota_b[:], in_=iota_i[:])
            ones = cp.tile([128, 1], BF16)
            nc.vector.memset(ones[:], 1.0)
            idx_sb = cp.tile([128, nsub], I32)
            nc.sync.dma_start(idx_sb[:], src_idx[:])
            dl_sb = cp.tile([128, nsub], F32)
            nc.sync.dma_start(dl_sb[:], dst_loc[:])
            al_sb = cp.tile([128, nsub], F32)
            nc.sync.dma_start(al_sb[:], alpha[:])
            asl_sb = cp.tile([128, NBLK], F32)
            nc.sync.dma_start(asl_sb[:], alphasl[:])
            # own-shard table rows (self-loop h): row (p*GBLK + t) for t<NBLK
            ta = tab[:]
            own_ap = bass.AP(ta.tensor, 0, [[GBLK * TC, 128], [TC, NBLK], [1, TC]])
            own_sb = cp.tile([128, NBLK * TC], BF16)
            nc.sync.dma_start(own_sb[:], own_ap)
            # wsl = exp(lrelu(alphasl))
            lsl_sb = cp.tile([128, NBLK], F32)
            nc.vector.tensor_scalar(out=lsl_sb[:], in0=asl_sb[:], scalar1=NEG,
                                    scalar2=None, op0=mybir.AluOpType.mult)
            nc.vector.tensor_tensor(out=lsl_sb[:], in0=lsl_sb[:], in1=asl_sb[:],
                                    op=mybir.AluOpType.max)
            wsl_sb = cp.tile([128, NBLK], F32)
            nc.scalar.activation(out=wsl_sb[:], in_=lsl_sb[:],
                                 func=mybir.ActivationFunctionType.Exp)
            # w = exp(lrelu(alpha)) in bulk: lrelu(x) = max(x, 0.2x)
            lr_sb = cp.tile([128, nsub], F32)
            nc.vector.tensor_scalar(out=lr_sb[:], in0=al_sb[:], scalar1=NEG,
                                    scalar2=None, op0=mybir.AluOpType.mult)
            nc.vector.tensor_tensor(out=lr_sb[:], in0=lr_sb[:], in1=al_sb[:],
                                    op=mybir.AluOpType.max)
            w_sb = cp.tile([128, nsub], F32)
            nc.scalar.activation(out=w_sb[:], in_=lr_sb[:],
                                 func=mybir.ActivationFunctionType.Exp)

            ps_sum = pstp.tile([1, 128], F32, tag="sum")
            ps_sq = pstp.tile([1, 128], F32, tag="sq")

            # zero-init gather buffers (stale-NaN guard)
            for i in range(8):
                gi = gp.tile([128, TMAX * TC], BF16, tag="g", name=f"ginit{i}")
                nc.vector.memset(gi[:], 0.0)

            q0 = 0
            og = None
            for t in range(NBLK):
                T = int(t_counts[t])
                g = gp.tile([128, TMAX * TC], BF16, tag="g")
                for s in range(T):
                    cnt = int(last_cnt[t]) if s == T - 1 else 128
                    nc.gpsimd.indirect_dma_start(
                        out=g[0:cnt, s * TC:(s + 1) * TC], out_offset=None,
                        in_=tab[:],
                        in_offset=bass.IndirectOffsetOnAxis(
                            ap=idx_sb[0:cnt, q0 + s:q0 + s + 1], axis=0))
                ps = pbp.tile([128, TC], F32, tag="b")
                for s in range(T):
                    q = q0 + s
                    cnt = int(last_cnt[t]) if s == T - 1 else 128
                    sw = swp.tile([128, 128], BF16, tag="sw")
                    nc.vector.tensor_scalar(
                        out=sw[0:cnt, :], in0=iota_b[0:cnt, :],
                        scalar1=dl_sb[0:cnt, q:q + 1],
                        scalar2=w_sb[0:cnt, q:q + 1],
                        op0=mybir.AluOpType.is_equal, op1=mybir.AluOpType.mult)
                    nc.tensor.matmul(out=ps[:], lhsT=sw[0:cnt, :],
                                     rhs=g[0:cnt, s * TC:(s + 1) * TC],
                                     start=(s == 0), stop=(s == T - 1))
                k = t % GRP
                if k == 0:
                    og = ogp.tile([128, GRP * HID], BF16, tag="og")
                num = rcp.tile([128, HID], F32, tag="num")
                nc.vector.scalar_tensor_tensor(
                    out=num[:], in0=own_sb[:, t * TC:t * TC + HID],
                    scalar=wsl_sb[:, t:t + 1], in1=ps[:, 0:HID],
                    op0=mybir.AluOpType.mult, op1=mybir.AluOpType.add)
                den = rcp.tile([128, 1], F32, tag="den")
                nc.vector.tensor_tensor(out=den[:], in0=ps[:, HID:HID + 1],
                                        in1=wsl_sb[:, t:t + 1],
                                        op=mybir.AluOpType.add)
                rc = rcp.tile([128, 1], F32, tag="rc")
                nc.vector.reciprocal(out=rc[:], in_=den[:])
                nc.vector.tensor_scalar(
                    out=og[:, k * HID:(k + 1) * HID], in0=num[:],
                    scalar1=rc[:], scalar2=None, op0=mybir.AluOpType.mult)
                sq = sqp.tile([128, 128], BF16, tag="sq")
                nc.scalar.activation(out=sq[:], in_=og[:, k * HID:(k + 1) * HID],
                                     func=mybir.ActivationFunctionType.Square)
                nc.tensor.matmul(out=ps_sum[:], lhsT=ones[:],
                                 rhs=og[:, k * HID:(k + 1) * HID],
                                 start=(t == 0), stop=(t == NBLK - 1))
                nc.tensor.matmul(out=ps_sq[:], lhsT=ones[:], rhs=sq[:],
                                 start=(t == 0), stop=(t == NBLK - 1))
                if k == GRP - 1:
                    g0 = t - (GRP - 1)
                    nc.scalar.dma_start(agg[:, g0 * HID:(t + 1) * HID], og[:])
                q0 += T

            st = cp.tile([1, 256], F32)
            nc.vector.tensor_copy(out=st[:, 0:128], in_=ps_sum[:])
            nc.vector.tensor_copy(out=st[:, 128:256], in_=ps_sq[:])
            nc.sync.dma_start(stats[:], st[:])
    nc.compile()
    return nc


# ---------------------------------------------------------------- node tail
def build_node2(classifier):
    """BN apply + relu + matmul (next-layer table or classifier)."""
    nc = bacc.Bacc("TRN2", target_bir_lowering=False, debug=False, num_devices=NCORE)
    agg = nc.dram_tensor("agg", [128, NBLK * HID], BF16, kind="ExternalInput")
    parts = nc.dram_tensor("parts", [8, 256], F32, kind="ExternalInput")
    gb = nc.dram_tensor("gb", [1, 256], F32, kind="ExternalInput")  # [gamma|beta]
    if classifier:
        Wn = nc.dram_tensor("Wn", [128, NCLS], BF16, kind="ExternalInput")
        bc = nc.dram_tensor("bc", [1, NCLS], F32, kind="ExternalInput")
        out = nc.dram_tensor("out", [128, NBLK * NCLS], F32, kind="ExternalOutput")
        ncols = NCLS
    else:
        Wn = nc.dram_tensor("Wn", [128, HID + 2], BF16, kind="ExternalInput")
        tab = nc.dram_tensor("tab", [128, NBLK * TC], BF16, kind="ExternalOutput")
        av = nc.dram_tensor("av", [128, NBLK * 2], F32, kind="ExternalOutput")
        ncols = HID + 2

    with tile.TileContext(nc) as tc:
        with (
            tc.tile_pool(name="c", bufs=1) as cp,
            tc.tile_pool(name="x", bufs=4) as xp,
            tc.tile_pool(name="o", bufs=3) as op,
            tc.tile_pool(name="ps", bufs=4, space="PSUM") as pp,
            tc.tile_pool(name="pt", bufs=2, space="PSUM") as ptp,
            tc.tile_pool(name="p1", bufs=2, space="PSUM") as p1p,
        ):
            identb = cp.tile([128, 128], BF16)
            make_identity(nc, identb[:])
            w_sb = cp.tile([128, ncols], BF16)
            nc.sync.dma_start(w_sb[:], Wn[:])
            agg_sb = cp.tile([128, NBLK * HID], BF16)
            nc.sync.dma_start(agg_sb[:], agg[:])

            # BN stats: mean/var from 8-core partials
            parts_sb = cp.tile([8, 256], F32)
            nc.sync.dma_start(parts_sb[:], parts[:])
            ones8 = cp.tile([8, 1], F32)
            nc.vector.memset(ones8[:], 1.0)
            st_ps = p1p.tile([1, 256], F32, tag="t1")
            nc.tensor.matmul(out=st_ps[:], lhsT=ones8[:], rhs=parts_sb[:],
                             start=True, stop=True)
            stat = cp.tile([1, 256], F32)
            nc.vector.tensor_scalar(out=stat[:], in0=st_ps[:], scalar1=1.0 / N,
                                    scalar2=None, op0=mybir.AluOpType.mult)
            mean = stat[:, 0:128]
            msq = stat[:, 128:256]
            var = cp.tile([1, 128], F32)
            nc.vector.tensor_tensor(out=var[:], in0=mean, in1=mean,
                                    op=mybir.AluOpType.mult)
            nc.vector.tensor_tensor(out=var[:], in0=msq, in1=var[:],
                                    op=mybir.AluOpType.subtract)
            nc.vector.tensor_scalar(out=var[:], in0=var[:], scalar1=EPS,
                                    scalar2=None, op0=mybir.AluOpType.add)
            std = cp.tile([1, 128], F32)
            nc.scalar.activation(out=std[:], in_=var[:],
                                 func=mybir.ActivationFunctionType.Sqrt)
            istd = cp.tile([1, 128], F32)
            nc.vector.reciprocal(out=istd[:], in_=std[:])
            gb_sb = cp.tile([1, 256], F32)
            nc.sync.dma_start(gb_sb[:], gb[:])
            gam = cp.tile([1, 128], F32)
            nc.vector.tensor_tensor(out=gam[:], in0=gb_sb[:, 0:128], in1=istd[:],
                                    op=mybir.AluOpType.mult)
            bet = cp.tile([1, 128], F32)
            nc.vector.tensor_tensor(out=bet[:], in0=mean, in1=gam[:],
                                    op=mybir.AluOpType.mult)
            nc.vector.tensor_tensor(out=bet[:], in0=gb_sb[:, 128:256], in1=bet[:],
                                    op=mybir.AluOpType.subtract)
            # gamma'/beta' as per-partition columns via K=1 matmul transpose
            one1 = cp.tile([1, 1], F32)
            nc.vector.memset(one1[:], 1.0)
            gcol_ps = p1p.tile([128, 1], F32, tag="t1")
            nc.tensor.matmul(out=gcol_ps[:], lhsT=gam[:], rhs=one1[:],
                             start=True, stop=True)
            gcol = cp.tile([128, 1], F32)
            nc.vector.tensor_copy(out=gcol[:], in_=gcol_ps[:])
            bcol_ps = p1p.tile([128, 1], F32, tag="t1")
            nc.tensor.matmul(out=bcol_ps[:], lhsT=bet[:], rhs=one1[:],
                             start=True, stop=True)
            bcol = cp.tile([128, 1], F32)
            nc.vector.tensor_copy(out=bcol[:], in_=bcol_ps[:])

            if classifier:
                bc_sb = cp.tile([1, NCLS], F32)
                nc.sync.dma_start(bc_sb[:], bc[:])
                one1b = cp.tile([1, 128], F32)
                nc.vector.memset(one1b[:], 1.0)
                bcb_ps = p1p.tile([128, NCLS], F32, tag="t1")
                nc.tensor.matmul(out=bcb_ps[:], lhsT=one1b[:], rhs=bc_sb[:],
                                 start=True, stop=True)
                bcb = cp.tile([128, NCLS], F32)
                nc.vector.tensor_copy(out=bcb[:], in_=bcb_ps[:])
            else:
                avst = cp.tile([128, NBLK * 2], F32)

            OW = NCLS if classifier else TC
            for g0 in range(0, NBLK, GRP):
                if classifier:
                    ot = op.tile([128, GRP * NCLS], F32, tag="o")
                else:
                    ot = op.tile([128, GRP * TC], BF16, tag="o")
                    nc.vector.memset(ot[:], 1.0)
                tr_g = ptp.tile([128, GRP * 128], BF16, tag="tr")
                for k in range(GRP):
                    t = g0 + k
                    nc.tensor.transpose(out=tr_g[:, k * 128:(k + 1) * 128],
                                        in_=agg_sb[:, t * HID:(t + 1) * HID],
                                        identity=identb[:])
                x2g = xp.tile([128, GRP * 128], BF16, tag="x2")
                nc.scalar.activation(out=x2g[:], in_=tr_g[:],
                                     func=mybir.ActivationFunctionType.Relu,
                                     bias=bcol[:], scale=gcol[:])
                if classifier:
                    hg = pp.tile([128, GRP * NCLS], F32, tag="h")
                    for k in range(GRP):
                        nc.tensor.matmul(out=hg[:, k * NCLS:(k + 1) * NCLS],
                                         lhsT=x2g[:, k * 128:(k + 1) * 128],
                                         rhs=w_sb[:], start=True, stop=True)
                    ba = bcb[:]
                    bc3 = bass.AP(ba.tensor, ba.offset,
                                  [list(ba.ap[0]), [0, GRP], [1, NCLS]])
                    nc.vector.tensor_tensor(out=ot[:], in0=hg[:], in1=bc3,
                                            op=mybir.AluOpType.add)
                else:
                    for k0, ln in ((0, 3), (3, 2), (5, 2)):
                        hg = pp.tile([128, 3 * ncols], F32, tag="h")
                        for j in range(ln):
                            nc.tensor.matmul(
                                out=hg[:, j * ncols:(j + 1) * ncols],
                                lhsT=x2g[:, (k0 + j) * 128:(k0 + j + 1) * 128],
                                rhs=w_sb[:], start=True, stop=True)
                        ha = hg[:]
                        oa = ot[:]
                        src3 = bass.AP(ha.tensor, ha.offset,
                                       [list(ha.ap[0]), [ncols, ln], [1, HID]])
                        dst3 = bass.AP(oa.tensor, oa.offset + (k0 * TC),
                                       [list(oa.ap[0]), [TC, ln], [1, HID]])
                        if k0 % 2 == 0:
                            nc.scalar.activation(
                                out=dst3, in_=src3,
                                func=mybir.ActivationFunctionType.Copy)
                        else:
                            nc.vector.tensor_copy(out=dst3, in_=src3)
                        sav = bass.AP(ha.tensor, ha.offset + HID,
                                      [list(ha.ap[0]), [ncols, ln], [1, 2]])
                        nc.vector.tensor_copy(
                            out=avst[:, (g0 + k0) * 2:(g0 + k0 + ln) * 2],
                            in_=sav)
                dst = out if classifier else tab
                nc.scalar.dma_start(dst[:, g0 * OW:(g0 + GRP) * OW], ot[:])
            if not classifier:
                nc.sync.dma_start(av[:], avst[:])
    nc.compile()
    return nc


# ---------------------------------------------------------------- host glue
def _edge_arrays(src, dst):
    """Per-core src_idx/dst_local/position arrays + shared t_counts.

    src_idx values address the p-major rolled table of the owning core:
      row(g) = (g % 128) * GBLK + ((g // 128 - NBLK*c) % GBLK)
    """
    order = np.argsort(dst, kind="stable")
    srcs = dst_sorted_src = src[order]
    dsts = dst[order]
    blk = dsts // BLK
    counts = np.bincount(blk, minlength=GBLK)
    starts = np.concatenate([[0], np.cumsum(counts)])
    cnt_mat = counts.reshape(NCORE, NBLK)
    t_counts = np.maximum(
        np.ceil(cnt_mat / BLK).astype(np.int64).max(axis=0), 1)
    last_cnt = np.maximum(
        (cnt_mat - (t_counts - 1)[None, :] * BLK).max(axis=0), 2)
    nsub = int(t_counts.sum())
    offs = np.concatenate([[0], np.cumsum(t_counts)])
    cores = []
    for c in range(NCORE):
        si = np.zeros((128, nsub), np.int32)
        dlv = np.full((128, nsub), 200.0, np.float32)
        pos_p, pos_q, e_s, e_d = [], [], [], []
        for t in range(NBLK):
            b = c * NBLK + t
            s0, e0 = int(starts[b]), int(starts[b + 1])
            cnt = e0 - s0
            if cnt == 0:
                continue
            k = np.arange(cnt)
            p = k % 128
            q = offs[t] + k // 128
            sg = srcs[s0:e0]
            dg = dsts[s0:e0]
            si[p, q] = ((sg % BLK) * GBLK
                        + (sg // BLK - NBLK * c) % GBLK).astype(np.int32)
            dlv[p, q] = (dg - b * BLK).astype(np.float32)
            pos_p.append(p)
            pos_q.append(q)
            e_s.append(sg)
            e_d.append(dg)
        cores.append(dict(
            si=si, dl=dlv,
            pp=np.concatenate(pos_p), pq=np.concatenate(pos_q),
            es=np.concatenate(e_s), ed=np.concatenate(e_d)))
    return t_counts, last_cnt, nsub, cores


def _assemble_tables(tab_outs):
    """Per-core L-launch table shards [128, NBLK*TC] -> rolled p-major
    full tables [NPAD, TC] per core."""
    tab3 = np.empty((128, GBLK, TC), dtype=BF)
    for c in range(NCORE):
        tab3[:, c * NBLK:(c + 1) * NBLK, :] = \
            tab_outs[c].reshape(128, NBLK, TC)
    tabs = []
    for c in range(NCORE):
        r = np.roll(tab3, -NBLK * c, axis=1)
        tabs.append(np.ascontiguousarray(r).reshape(NPAD, TC))
    return tabs


def _assemble_av(av_outs):
    """Per-core av shards [128, NBLK*2] f32 -> global asv/adv [NPAD]."""
    asv = np.empty(NPAD, np.float32)
    adv = np.empty(NPAD, np.float32)
    for c in range(NCORE):
        a3 = av_outs[c].reshape(128, NBLK, 2)
        asv.reshape(GBLK, 128)[c * NBLK:(c + 1) * NBLK] = a3[:, :, 0].T
        adv.reshape(GBLK, 128)[c * NBLK:(c + 1) * NBLK] = a3[:, :, 1].T
    return asv, adv


def _alphasl_arrays(asv, adv):
    """Self-loop alpha per core: [128, NBLK] f32, node (t,p) of core c."""
    outs = []
    a = (asv + adv).reshape(NCORE, NBLK, 128)
    for c in range(NCORE):
        outs.append(np.ascontiguousarray(a[c].T))
    return outs


def _alpha_arrays(asv, adv, cores, nsub):
    outs = []
    for core in cores:
        A = np.full((128, nsub), PAD_ALPHA, np.float32)
        A[core['pp'], core['pq']] = asv[core['es']] + adv[core['ed']]
        outs.append(A)
    return outs


_CACHE = {}


def kernel(x, edge_index, W1, as1, ad1, b1, g1, beta1,
           W2, as2, ad2, b2, g2, beta2, Wc, bc):
    x = np.asarray(x, np.float32)
    ei = np.asarray(edge_index)
    src = ei[0].astype(np.int64)
    dst = ei[1].astype(np.int64)

    t_counts, last_cnt, nsub, cores = _edge_arrays(src, dst)
    key = tuple(t_counts.tolist()) + tuple(last_cnt.tolist())
    if key not in _CACHE:
        _CACHE[key] = (build_l1(), build_edge(t_counts, last_cnt),
                       build_node2(False), build_node2(True))
    nc1, nce, nc3, nc5 = _CACHE[key]

    W1 = np.asarray(W1, np.float32)
    W2 = np.asarray(W2, np.float32)
    Wc = np.asarray(Wc, np.float32)
    g1 = np.asarray(g1, np.float32)
    beta1 = np.asarray(beta1, np.float32)
    g2 = np.asarray(g2, np.float32)
    beta2 = np.asarray(beta2, np.float32)

    # ---- L1
    xp = np.zeros((NPAD, 128), np.float32)
    xp[:N] = x
    waug1 = np.concatenate([W1, (W1 @ np.asarray(as1, np.float32))[:, None],
                            (W1 @ np.asarray(ad1, np.float32))[:, None]],
                           axis=1).astype(BF)
    in1 = [{"xT": np.ascontiguousarray(xp[c * PC:(c + 1) * PC].T).astype(BF),
            "waug": waug1} for c in range(NCORE)]
    r1 = _run(nc1, in1, "L1")

    tabs1 = _assemble_tables([r1[c]["tab"] for c in range(NCORE)])
    asv1, adv1 = _assemble_av([r1[c]["av"] for c in range(NCORE)])
    alphas1 = _alpha_arrays(asv1, adv1, cores, nsub)
    asl1 = _alphasl_arrays(asv1, adv1)

    # ---- E1
    ine = [{"tab": tabs1[c], "src_idx": cores[c]['si'],
            "dst_loc": cores[c]['dl'], "alpha": alphas1[c],
            "alphasl": asl1[c]}
           for c in range(NCORE)]
    re1 = _run(nce, ine, "E1")
    agg1 = [re1[c]["agg"] for c in range(NCORE)]
    parts1 = np.stack([re1[c]["stats"][0] for c in range(NCORE)], axis=0)

    # ---- L3
    gb1 = np.concatenate([g1, beta1])[None, :]
    waug2 = np.concatenate([W2, (W2 @ np.asarray(as2, np.float32))[:, None],
                            (W2 @ np.asarray(ad2, np.float32))[:, None]],
                           axis=1).astype(BF)
    in3 = [{"agg": agg1[c], "parts": parts1, "gb": gb1, "Wn": waug2}
           for c in range(NCORE)]
    r3 = _run(nc3, in3, "L3")

    tabs2 = _assemble_tables([r3[c]["tab"] for c in range(NCORE)])
    asv2, adv2 = _assemble_av([r3[c]["av"] for c in range(NCORE)])
    alphas2 = _alpha_arrays(asv2, adv2, cores, nsub)
    asl2 = _alphasl_arrays(asv2, adv2)

    # ---- E2
    ine2 = [{"tab": tabs2[c], "src_idx": cores[c]['si'],
             "dst_loc": cores[c]['dl'], "alpha": alphas2[c],
             "alphasl": asl2[c]}
            for c in range(NCORE)]
    re2 = _run(nce, ine2, "E2")
    agg2 = [re2[c]["agg"] for c in range(NCORE)]
    parts2 = np.stack([re2[c]["stats"][0] for c in range(NCORE)], axis=0)

    # pad-node correction for BN2 stats: pad nodes aggregate to
    # x2_pad = relu(BN1(0)) and h2_pad = x2_pad @ W2, included (NPAD - N)
    # times in the E2 partial sums but absent from the reference mean.
    s1 = parts1.sum(axis=0)
    mu1 = s1[0:128] / N
    var1 = s1[128:256] / N - mu1 * mu1
    x2pad = np.maximum((0.0 - mu1) / np.sqrt(var1 + EPS) * g1 + beta1, 0.0)
    h2pad = x2pad.astype(BF).astype(np.float32) @ \
        waug2[:, 0:HID].astype(np.float32)
    npad_extra = NPAD - N
    parts2 = parts2.copy()
    parts2[0, 0:128] -= npad_extra * h2pad
    parts2[0, 128:256] -= npad_extra * h2pad * h2pad

    # ---- L5
    gb2 = np.concatenate([g2, beta2])[None, :]
    in5 = [{"agg": agg2[c], "parts": parts2, "gb": gb2,
            "Wn": Wc.astype(BF),
            "bc": np.asarray(bc, np.float32)[None, :]} for c in range(NCORE)]
    r5 = _run(nc5, in5, "L5")

    logits = np.empty((NPAD, NCLS), np.float32)
    l4 = logits.reshape(NCORE, NBLK, 128, NCLS)
    for c in range(NCORE):
        l4[c] = r5[c]["out"].reshape(128, NBLK, NCLS).transpose(1, 0, 2)
    return logits[:N]
